# revision 1
# baseline (speedup 1.0000x reference)
"""Trainium2 Bass kernel for nn_DownModel (GNN message passing + kNN graph + GCN).

Math (from the reference):
  f1   = elu(c00*global*relu(pre_token*feat) + c01*pre_token_w*feat)      [N,H]
  agg  = scatter-add over E edges of adj_vals * f1[adj_cols]              [N,H]
  y    = f1 @ gcn_w                                                       [N,C]
  emb  = normalize_rows(balance_w * [f1 | agg])                           [N,2H]
  sim  = emb @ emb.T ; vals,idx = top_k(sim, 16)                          [N,N]
  out  = relu(0.5*(A @ y) + 0.5*sum_j relu(vals_j)*y[idx_j] + b)          [N,C]

Sharding: row-block N across 8 cores.  Each core computes f1/y for its rows,
all-gathers the [N, H+C] table, does the edge segment-sum for its rows via
one-hot-selector matmuls (edges bucketed host-side by destination row chunk),
all-gathers the transposed embedding, computes its [N/8, N] block of sim,
top-k's each row on the vector engine, and combines.  new_adj is never
materialized: new_adj @ y = 0.5*SpMM_edges(y) + 0.5*topk-weighted-gather(y).
"""

import numpy as np

import concourse.bass as bass
import concourse.mybir as mybir
import concourse.tile as tile
from concourse.bass_utils import run_bass_kernel_spmd

F32 = mybir.dt.float32
I32 = mybir.dt.int32
U32 = mybir.dt.uint32

N = 8192
H = 256
C_OUT = 40
K_TOP = 16
NCORES = 8
P = 128
ROWS_PER_CORE = N // NCORES          # 1024
CHUNKS = ROWS_PER_CORE // P          # 8
KT = (2 * H) // P                    # 4 k-tiles of the 2H embedding dim
HKT = H // P                         # 2 k-tiles of the H dim
TBLW = H + C_OUT                     # 296 table row width
SIMW = 512                           # sim column chunk width (one PSUM bank)
NCC = N // SIMW                      # 16 column chunks
GROUP = 4                            # row-tiles sharing one rhs stream pass


def _split_waits(nc, maxw=1):
    """This container's walrus only accepts one sync-wait command per
    instruction; hoist excess waits onto preceding same-engine NOPs."""
    n_new = 0
    for bb in nc.main_func.blocks:
        new_insts = []
        for ins in bb.instructions:
            si = ins.sync_info
            if si is not None and si.on_wait and len(si.on_wait) > maxw:
                waits = list(si.on_wait)
                excess, keep = waits[:-maxw], waits[-maxw:]
                for i in range(0, len(excess), maxw):
                    nop = mybir.InstNoOp(
                        name=f"waitnop-{ins.name}-{i}",
                        engine=ins.engine,
                        ins=[],
                        outs=[],
                        sync_info=mybir.SyncInfo(
                            on_wait=excess[i:i + maxw], on_update=[]
                        ),
                    )
                    new_insts.append(nop)
                    n_new += 1
                si.on_wait = keep
            new_insts.append(ins)
        bb.instructions[:] = new_insts
    return n_new


def build(T, sim_dtype=F32, n=N, debug=False, repeat=1, phase_lim=6):
    """Build the SPMD program (identical on all cores; data differs).

    T: number of 128-edge tiles per destination row chunk (padded, global max).
    Execution cost here is dominated by per-instruction dispatch, so ops are
    batched across row-tiles/edge-tiles wherever the ISA allows.
    """
    rows_per_core = n // NCORES
    chunks = rows_per_core // P
    ncc = n // SIMW
    ccg_n = 4                    # sim column chunks per PSUM macro-tile
    nccg = ncc // ccg_n
    ngroups = (chunks + GROUP - 1) // GROUP

    nc = bass.Bass(num_devices=NCORES)
    if debug:
        dbg_table = nc.dram_tensor("dbg_table", [n, TBLW], F32,
                                   kind="ExternalOutput")
        dbg_agg = nc.dram_tensor("dbg_agg", [P, chunks * TBLW], F32,
                                 kind="ExternalOutput")
        dbg_embT = nc.dram_tensor("dbg_embT", [NCORES * 2 * H, rows_per_core],
                                  F32, kind="ExternalOutput")
        dbg_m16 = nc.dram_tensor("dbg_m16", [P, chunks * K_TOP], F32,
                                 kind="ExternalOutput")
        dbg_i16 = nc.dram_tensor("dbg_i16", [P, chunks * K_TOP], U32,
                                 kind="ExternalOutput")

    # ---- per-core inputs ----
    feat_d = nc.dram_tensor("feat", [rows_per_core, H], F32, kind="ExternalInput")
    erow_d = nc.dram_tensor("erow", [chunks, P, T], F32, kind="ExternalInput")
    ecol_d = nc.dram_tensor("ecol", [chunks, P, T], I32, kind="ExternalInput")
    eval_d = nc.dram_tensor("eval", [chunks, P, T], F32, kind="ExternalInput")
    # replicated small params (host pre-broadcast across partitions)
    cA_d = nc.dram_tensor("cA", [P, H], F32, kind="ExternalInput")
    cB_d = nc.dram_tensor("cB", [P, H], F32, kind="ExternalInput")
    cC_d = nc.dram_tensor("cC", [P, H], F32, kind="ExternalInput")
    bal_d = nc.dram_tensor("bal", [P, 2 * H], F32, kind="ExternalInput")
    bias_d = nc.dram_tensor("bias", [P, C_OUT], F32, kind="ExternalInput")
    gcnw_d = nc.dram_tensor("gcnw", [H, C_OUT], F32, kind="ExternalInput")
    iota_d = nc.dram_tensor("iota", [P, P], F32, kind="ExternalInput")
    ident_d = nc.dram_tensor("ident", [P, P], F32, kind="ExternalInput")

    out_d = nc.dram_tensor("out", [rows_per_core, C_OUT], F32, kind="ExternalOutput")

    with tile.TileContext(nc) as tc:
        with tc.tile_pool(name="consts", bufs=1) as cp, \
             tc.tile_pool(name="persist", bufs=1) as pp, \
             tc.tile_pool(name="dram", bufs=1, space="DRAM") as dp:

            # ---- constants into SBUF ----
            cA = cp.tile([P, H], F32)
            cB = cp.tile([P, H], F32)
            cC = cp.tile([P, H], F32)
            bal = cp.tile([P, 2 * H], F32)
            bias = cp.tile([P, C_OUT], F32)
            gcnw = cp.tile([P, HKT * C_OUT], F32)
            iota = cp.tile([P, P], F32)
            ident = cp.tile([P, P], F32)
            nc.sync.dma_start(out=cA[:], in_=cA_d[:])
            nc.sync.dma_start(out=cB[:], in_=cB_d[:])
            nc.sync.dma_start(out=cC[:], in_=cC_d[:])
            nc.sync.dma_start(out=bal[:], in_=bal_d[:])
            nc.sync.dma_start(out=bias[:], in_=bias_d[:])
            for k in range(HKT):
                nc.sync.dma_start(out=gcnw[:, k * C_OUT:(k + 1) * C_OUT],
                                  in_=gcnw_d[k * P:(k + 1) * P, :])
            nc.sync.dma_start(out=iota[:], in_=iota_d[:])
            nc.sync.dma_start(out=ident[:], in_=ident_d[:])

            # ---- persistent SBUF tensors ----
            agg_all = pp.tile([P, chunks * TBLW], F32)      # [agg | A@y] per chunk
            embTloc = pp.tile([P, KT * rows_per_core], F32)  # local emb, transposed

            for rep in range(repeat):
                # ---- DRAM scratch (per rep: Shared tensors are
                # single-writer) ----
                table_loc = dp.tile([rows_per_core, TBLW], F32,
                                    name=f"table_loc_{rep}")
                table_g = dp.tile([n, TBLW], F32, addr_space="Shared",
                                  name=f"table_g_{rep}")
                embT_loc_d = dp.tile([2 * H, rows_per_core], F32,
                                     name=f"embT_loc_d_{rep}")
                embT_g = dp.tile([NCORES * 2 * H, rows_per_core], F32,
                                 addr_space="Shared", name=f"embT_g_{rep}")

                # f1 lives only through P3; manual pool scope frees its
                # SBUF before the P4 sim buffers open.
                _f1ctx = tc.tile_pool(name=f"f1p_{rep}", bufs=1)
                f1p = _f1ctx.__enter__()
                f1_all = f1p.tile([P, chunks * H], F32, name=f"f1_all_{rep}")
                # ===== P1: f1 + y for the local row block (batched) =====
                with tc.tile_pool(name=f"p1_{rep}", bufs=1) as p1, \
                     tc.tile_pool(name=f"p1s_{rep}", bufs=3) as p1s, \
                     tc.tile_pool(name=f"p1ps_{rep}", bufs=2, space="PSUM") as p1ps:
                    W1 = chunks * H
                    ft = p1.tile([P, W1], F32)
                    nc.sync.dma_start(
                        out=ft[:].rearrange("p (r w) -> p r w", r=chunks),
                        in_=feat_d[:].rearrange("(r p) w -> p r w", p=P))
                    mB = lambda c: c[:, None, :].to_broadcast([P, chunks, H])
                    v3 = lambda t: t[:].rearrange("p (r w) -> p r w", r=chunks)
                    m = p1.tile([P, W1], F32)
                    nc.vector.tensor_tensor(out=v3(m), in0=v3(ft), in1=mB(cB),
                                            op=mybir.AluOpType.mult)
                    nc.vector.tensor_scalar(out=m[:], in0=m[:], scalar1=0.0,
                                            scalar2=None, op0=mybir.AluOpType.max)
                    nc.vector.tensor_tensor(out=v3(m), in0=v3(m), in1=mB(cA),
                                            op=mybir.AluOpType.mult)
                    v = p1.tile([P, W1], F32)
                    nc.vector.tensor_tensor(out=v3(v), in0=v3(ft), in1=mB(cC),
                                            op=mybir.AluOpType.mult)
                    nc.vector.tensor_tensor(out=m[:], in0=m[:], in1=v[:],
                                            op=mybir.AluOpType.add)
                    # elu(z) = (relu(z) - 1) + exp(min(z, 0))
                    nc.vector.tensor_scalar(out=v[:], in0=m[:], scalar1=0.0,
                                            scalar2=-1.0, op0=mybir.AluOpType.max,
                                            op1=mybir.AluOpType.add)
                    nc.vector.tensor_scalar(out=m[:], in0=m[:], scalar1=0.0,
                                            scalar2=None, op0=mybir.AluOpType.min)
                    nc.scalar.activation(out=m[:], in_=m[:],
                                         func=mybir.ActivationFunctionType.Exp)
                    nc.vector.tensor_tensor(out=f1_all[:], in0=v[:], in1=m[:],
                                            op=mybir.AluOpType.add)
                    nc.sync.dma_start(
                        out=table_loc[:, 0:H].rearrange("(r p) w -> p r w", p=P),
                        in_=v3(f1_all))
                    # y = f1 @ gcn_w  (transpose f1 k-blocks, then matmul;
                    # all row-tiles accumulate into one PSUM bank)
                    psy = p1ps.tile([P, chunks * C_OUT], F32, space="PSUM",
                                    name="psy_all")
                    for rt in range(chunks):
                        f1T = p1s.tile([P, HKT * P], F32)
                        psT = p1ps.tile([P, HKT * P], F32, space="PSUM")
                        for k in range(HKT):
                            nc.tensor.transpose(
                                out=psT[:, k * P:(k + 1) * P],
                                in_=f1_all[:, rt * H + k * P:rt * H + (k + 1) * P],
                                identity=ident[:])
                        nc.scalar.copy(out=f1T[:], in_=psT[:])
                        for k in range(HKT):
                            nc.tensor.matmul(out=psy[:, rt * C_OUT:(rt + 1) * C_OUT],
                                             lhsT=f1T[:, k * P:(k + 1) * P],
                                             rhs=gcnw[:, k * C_OUT:(k + 1) * C_OUT],
                                             start=(k == 0), stop=(k == HKT - 1))
                    yt = p1s.tile([P, chunks * C_OUT], F32, name="yt_all")
                    nc.scalar.copy(out=yt[:], in_=psy[:])
                    nc.sync.dma_start(
                        out=table_loc[:, H:TBLW].rearrange("(r p) w -> p r w", p=P),
                        in_=yt[:].rearrange("p (r w) -> p r w", r=chunks))

                if phase_lim >= 2:
                    # ===== P1b: all-gather the [n, H+C] table =====
                    nc.gpsimd.collective_compute(
                        "AllGather", mybir.AluOpType.bypass,
                        replica_groups=[list(range(NCORES))],
                        ins=[table_loc.opt()], outs=[table_g.opt()],
                    )
                    if debug and rep == 0:
                        nc.sync.dma_start(out=dbg_table[:], in_=table_g[:])

                if phase_lim >= 3:
                    # ===== P2: edge segment-sum (agg | A@y) =====
                    with tc.tile_pool(name=f"p2e_{rep}", bufs=4) as p2e, \
                         tc.tile_pool(name=f"p2g_{rep}", bufs=2) as p2g, \
                         tc.tile_pool(name=f"p2s_{rep}", bufs=2) as p2s, \
                         tc.tile_pool(name=f"p2ps_{rep}", bufs=2, space="PSUM") as p2ps:
                        for ci in range(chunks):
                            er = p2e.tile([P, T], F32)
                            ec = p2e.tile([P, T], I32)
                            ev = p2e.tile([P, T], F32)
                            nc.sync.dma_start(out=er[:], in_=erow_d[ci])
                            nc.sync.dma_start(out=ec[:], in_=ecol_d[ci])
                            nc.sync.dma_start(out=ev[:], in_=eval_d[ci])
                            # one-hot selector blocks for all T tiles: 2 ops
                            S_all = p2s.tile([P, T * P], F32)
                            S3 = S_all[:].rearrange("p (t r) -> p t r", t=T)
                            nc.vector.tensor_tensor(
                                out=S3,
                                in0=er[:, :, None].to_broadcast([P, T, P]),
                                in1=iota[:, None, :].to_broadcast([P, T, P]),
                                op=mybir.AluOpType.is_equal)
                            nc.vector.tensor_tensor(
                                out=S3, in0=S3,
                                in1=ev[:, :, None].to_broadcast([P, T, P]),
                                op=mybir.AluOpType.mult)
                            psa = p2ps.tile([P, TBLW], F32, space="PSUM")
                            gb = p2g.tile([P, T * TBLW], F32, tag="gtile",
                                          name=f"g_{ci}")
                            for t in range(T):
                                nc.gpsimd.indirect_dma_start(
                                    out=gb[:, t * TBLW:(t + 1) * TBLW],
                                    out_offset=None,
                                    in_=table_g[:, :],
                                    in_offset=bass.IndirectOffsetOnAxis(
                                        ap=ec[:, t:t + 1], axis=0),
                                )
                            for t in range(T):
                                nc.tensor.matmul(out=psa[:],
                                                 lhsT=S_all[:, t * P:(t + 1) * P],
                                                 rhs=gb[:, t * TBLW:(t + 1) * TBLW],
                                                 start=(t == 0), stop=(t == T - 1))
                            nc.scalar.copy(out=agg_all[:, ci * TBLW:(ci + 1) * TBLW],
                                           in_=psa[:])
                    if debug and rep == 0:
                        nc.sync.dma_start(out=dbg_agg[:], in_=agg_all[:])

                if phase_lim >= 4:
                    # ===== P3: embedding build + transpose (batched) =====
                    with tc.tile_pool(name=f"p3_{rep}", bufs=1) as p3, \
                         tc.tile_pool(name=f"p3ps_{rep}", bufs=2, space="PSUM") as p3ps:
                        W3 = chunks * 2 * H
                        zc = p3.tile([P, W3], F32)
                        zc3 = zc[:].rearrange("p (r w) -> p r w", r=chunks)
                        nc.vector.tensor_tensor(
                            out=zc3[:, :, 0:H],
                            in0=f1_all[:].rearrange("p (r w) -> p r w", r=chunks),
                            in1=bal[:, None, 0:H].to_broadcast([P, chunks, H]),
                            op=mybir.AluOpType.mult)
                        nc.vector.tensor_tensor(
                            out=zc3[:, :, H:2 * H],
                            in0=agg_all[:].rearrange(
                                "p (r w) -> p r w", r=chunks)[:, :, 0:H],
                            in1=bal[:, None, H:2 * H].to_broadcast([P, chunks, H]),
                            op=mybir.AluOpType.mult)
                        sq = p3.tile([P, W3], F32)
                        n2 = p3.tile([P, chunks], F32)
                        for rt in range(chunks):
                            # Square + row-sum fused on the (idle) scalar
                            # engine; keeps the 4MB reduce off the DVE.
                            nc.scalar.activation(
                                out=sq[:, rt * 2 * H:(rt + 1) * 2 * H],
                                in_=zc[:, rt * 2 * H:(rt + 1) * 2 * H],
                                func=mybir.ActivationFunctionType.Square,
                                accum_out=n2[:, rt:rt + 1])
                        nc.scalar.sqrt(out=n2[:], in_=n2[:])
                        nc.vector.tensor_scalar(out=n2[:], in0=n2[:], scalar1=1e-8,
                                                scalar2=None, op0=mybir.AluOpType.add)
                        inv = p3.tile([P, chunks], F32)
                        nc.vector.reciprocal(out=inv[:], in_=n2[:])
                        nc.vector.tensor_tensor(
                            out=zc3, in0=zc3,
                            in1=inv[:, :, None].to_broadcast([P, chunks, 2 * H]),
                            op=mybir.AluOpType.mult)
                        for rt in range(chunks):
                            psT = p3ps.tile([P, KT * P], F32, space="PSUM")
                            for k in range(KT):
                                nc.tensor.transpose(
                                    out=psT[:, k * P:(k + 1) * P],
                                    in_=zc[:, rt * 2 * H + k * P:
                                           rt * 2 * H + (k + 1) * P],
                                    identity=ident[:])
                            dst3 = embTloc[:].rearrange(
                                "p (k r) -> p k r", k=KT)[:, :, rt * P:(rt + 1) * P]
                            nc.scalar.copy(out=dst3, in_=psT[:].rearrange(
                                "p (k r) -> p k r", k=KT))
                            nc.sync.dma_start(
                                out=embT_loc_d[:, rt * P:(rt + 1) * P].rearrange(
                                    "(k p) r -> p k r", p=P),
                                in_=dst3)

                    # ===== P3b: all-gather transposed embedding =====
                    nc.gpsimd.collective_compute(
                        "AllGather", mybir.AluOpType.bypass,
                        replica_groups=[list(range(NCORES))],
                        ins=[embT_loc_d.opt()], outs=[embT_g.opt()],
                    )
                    if debug and rep == 0:
                        nc.sync.dma_start(out=dbg_embT[:], in_=embT_g[:])

                _f1ctx.__exit__(None, None, None)

                if phase_lim >= 5:
                    # ===== P4+P5: sim row-block, top-k, combine =====
                    cpb = max(1, rows_per_core // SIMW)
                    bw = cpb * SIMW
                    nblk = ccg_n // cpb
                    with tc.tile_pool(name=f"p4rhs_{rep}", bufs=nblk) as p4rhs, \
                         tc.tile_pool(name=f"p4sim_{rep}", bufs=GROUP) as p4sim, \
                         tc.tile_pool(name=f"p4s_{rep}", bufs=2) as p4s, \
                         tc.tile_pool(name=f"p4ps_{rep}", bufs=2, space="PSUM") as p4ps:
                        ot_all = p4s.tile([P, chunks * C_OUT], F32,
                                          bufs=1, name=f"ot_all_{rep}")
                        for g in range(ngroups):
                            rts = [g * GROUP + j for j in range(GROUP)
                                   if g * GROUP + j < chunks]
                            sims = {}
                            for rt in rts:
                                sims[rt] = p4sim.tile([P, n], sim_dtype,
                                                      tag="simbuf",
                                                      name=f"sim_rt{rt}")
                            for cg in range(nccg):
                                halves = []
                                for hf in range(nblk):
                                    blk = cg * nblk + hf
                                    rh = p4rhs.tile([P, KT * bw],
                                                    sim_dtype, tag="rhs",
                                                    name=f"rhs{cg}_{hf}")
                                    nc.sync.dma_start(
                                        out=rh[:].rearrange(
                                            "p (k w) -> p k w", k=KT),
                                        in_=embT_g[blk * 2 * H:(blk + 1) * 2 * H,
                                                   0:bw].rearrange(
                                            "(k p) w -> p k w", p=P))
                                    halves.append(rh)
                                rhss = []
                                for ccq in range(ccg_n):
                                    rhss.append((halves[ccq // cpb], ccq % cpb))
                                for rt in rts:
                                    pss = p4ps.tile([P, ccg_n * SIMW], F32,
                                                    space="PSUM", tag="pss",
                                                    name=f"pss{rt}")
                                    for ccq in range(ccg_n):
                                        for k in range(KT):
                                            nc.tensor.matmul(
                                                out=pss[:, ccq * SIMW:
                                                        (ccq + 1) * SIMW],
                                                lhsT=embTloc[
                                                    :, k * rows_per_core + rt * P:
                                                    k * rows_per_core + (rt + 1) * P],
                                                rhs=rhss[ccq][0][
                                                :, k * bw
                                                + rhss[ccq][1] * SIMW:
                                                k * bw
                                                + (rhss[ccq][1] + 1) * SIMW],
                                                start=(k == 0), stop=(k == KT - 1))
                                    nc.scalar.copy(
                                        out=sims[rt][:, cg * ccg_n * SIMW:
                                                     (cg + 1) * ccg_n * SIMW],
                                        in_=pss[:])
                            for rt in rts:
                                if phase_lim < 6:
                                    mx = p4s.tile([P, 8], F32, name=f"mx{rt}")
                                    nc.vector.max(out=mx[:], in_=sims[rt][:])
                                    continue
                                sim = sims[rt]
                                m16 = p4s.tile([P, K_TOP], F32, tag="m16",
                                               bufs=chunks, name=f"m16_{rt}")
                                i16 = p4s.tile([P, K_TOP], U32, tag="i16",
                                               bufs=chunks, name=f"i16_{rt}")
                                nc.vector.max(out=m16[:, 0:8], in_=sim[:])
                                nc.vector.max_index(out=i16[:, 0:8],
                                                    in_max=m16[:, 0:8],
                                                    in_values=sim[:])
                                nc.vector.match_replace(out=sim[:],
                                                        in_to_replace=m16[:, 0:8],
                                                        in_values=sim[:],
                                                        imm_value=-1e30)
                                nc.vector.max(out=m16[:, 8:16], in_=sim[:])
                                nc.vector.max_index(out=i16[:, 8:16],
                                                    in_max=m16[:, 8:16],
                                                    in_values=sim[:])
                                if debug and rep == 0:
                                    nc.sync.dma_start(
                                        out=dbg_m16[:, rt * K_TOP:(rt + 1) * K_TOP],
                                        in_=m16[:])
                                    nc.sync.dma_start(
                                        out=dbg_i16[:, rt * K_TOP:(rt + 1) * K_TOP],
                                        in_=i16[:])
                                # P5: out2 = sum_j relu(v_j) * y[idx_j]
                                v16 = p4s.tile([P, K_TOP], F32, tag="v16",
                                               bufs=chunks, name=f"v16_{rt}")
                                nc.vector.tensor_scalar(out=v16[:], in0=m16[:],
                                                        scalar1=0.0, scalar2=None,
                                                        op0=mybir.AluOpType.max)
                                y16 = p4s.tile([P, K_TOP * C_OUT], F32)
                                for j in range(K_TOP):
                                    nc.gpsimd.indirect_dma_start(
                                        out=y16[:, j * C_OUT:(j + 1) * C_OUT],
                                        out_offset=None,
                                        in_=table_g[:, :],
                                        in_offset=bass.IndirectOffsetOnAxis(
                                            ap=i16[:, j:j + 1], axis=0),
                                        element_offset=H,
                                    )
                                y16s = p4s.tile([P, K_TOP * C_OUT], F32)
                                nc.scalar.copy(out=y16s[:], in_=y16[:])
                                nc.vector.tensor_tensor(
                                    out=y16s[:].rearrange("p (a b) -> p a b",
                                                          a=K_TOP),
                                    in0=y16s[:].rearrange("p (a b) -> p a b",
                                                          a=K_TOP),
                                    in1=v16[:, :, None].to_broadcast(
                                        [P, K_TOP, C_OUT]),
                                    op=mybir.AluOpType.mult)
                                nc.vector.tensor_reduce(
                                    out=ot_all[:, rt * C_OUT:(rt + 1) * C_OUT],
                                    in_=y16s[:].rearrange("p (a b) -> p b a",
                                                          a=K_TOP),
                                    axis=mybir.AxisListType.X,
                                    op=mybir.AluOpType.add)

                        if phase_lim >= 6:
                            # batched: out = relu(0.5*(out1 + out2) + bias)
                            o3 = ot_all[:].rearrange("p (r w) -> p r w", r=chunks)
                            nc.vector.tensor_tensor(
                                out=o3, in0=o3,
                                in1=agg_all[:].rearrange(
                                    "p (r w) -> p r w", r=chunks)[:, :, H:TBLW],
                                op=mybir.AluOpType.add)
                            nc.vector.tensor_scalar(out=ot_all[:], in0=ot_all[:],
                                                    scalar1=0.5, scalar2=None,
                                                    op0=mybir.AluOpType.mult)
                            nc.vector.tensor_tensor(
                                out=o3, in0=o3,
                                in1=bias[:, None, :].to_broadcast(
                                    [P, chunks, C_OUT]),
                                op=mybir.AluOpType.add)
                            nc.vector.tensor_scalar(out=ot_all[:], in0=ot_all[:],
                                                    scalar1=0.0, scalar2=None,
                                                    op0=mybir.AluOpType.max)
                            nc.sync.dma_start(
                                out=out_d[:].rearrange("(r p) w -> p r w", p=P),
                                in_=o3)

            if phase_lim < 6:
                with tc.tile_pool(name="dummyout", bufs=1) as dop:
                    zz = dop.tile([P, C_OUT], F32)
                    nc.vector.memset(zz[:], 0.0)
                    for rt in range(chunks):
                        nc.sync.dma_start(out=out_d[rt * P:(rt + 1) * P, :],
                                          in_=zz[:])

    return nc


def prep_inputs(features, adj_rows, adj_cols, adj_vals, tokens, wp_weight,
                global_token, pre_token_w, combine_w, balance_w, gcn_w, gcn_b,
                n=N):
    """Host-side sharding: row-block features, bucket edges by destination
    row chunk, pre-broadcast the small parameters."""
    rows_per_core = n // NCORES
    chunks = rows_per_core // P

    features = np.ascontiguousarray(np.asarray(features, dtype=np.float32))
    r = np.asarray(adj_rows).astype(np.int64)
    c = np.asarray(adj_cols).astype(np.int64)
    v = np.asarray(adj_vals, dtype=np.float32)

    pre_token = (np.asarray(wp_weight, np.float32) @
                 np.asarray(tokens, np.float32)).reshape(-1)       # [H]
    cw = np.asarray(combine_w, np.float32).reshape(-1)
    cA = (cw[0] * np.asarray(global_token, np.float32)).reshape(-1)
    cB = pre_token
    cC = (cw[1] * np.asarray(pre_token_w, np.float32)).reshape(-1)
    bal = np.asarray(balance_w, np.float32).reshape(-1)
    bias = np.asarray(gcn_b, np.float32).reshape(-1)

    bcast = lambda x: np.ascontiguousarray(np.tile(x[None, :], (P, 1)))
    cA_b, cB_b, cC_b = bcast(cA), bcast(cB), bcast(cC)
    bal_b, bias_b = bcast(bal), bcast(bias)
    gcnw = np.ascontiguousarray(np.asarray(gcn_w, np.float32))
    iota = np.tile(np.arange(P, dtype=np.float32)[None, :], (P, 1))
    ident = np.eye(P, dtype=np.float32)

    # bucket edges by (core, chunk); sort key = global chunk id
    gchunk = r // P                       # 0 .. n/P-1
    order = np.argsort(gchunk, kind="stable")
    rs, cs, vs = r[order], c[order], v[order]
    gs = gchunk[order]
    counts = np.bincount(gs, minlength=n // P)
    T = max(1, int(np.ceil(counts.max() / P)))

    erow = np.full((NCORES, chunks, T * P), -1.0, dtype=np.float32)
    ecol = np.zeros((NCORES, chunks, T * P), dtype=np.int32)
    evalv = np.zeros((NCORES, chunks, T * P), dtype=np.float32)
    starts = np.concatenate([[0], np.cumsum(counts)])
    for g in range(n // P):
        core, ci = g // chunks, g % chunks
        s, e = starts[g], starts[g + 1]
        cnt = e - s
        erow[core, ci, :cnt] = (rs[s:e] % P).astype(np.float32)
        ecol[core, ci, :cnt] = cs[s:e].astype(np.int32)
        evalv[core, ci, :cnt] = vs[s:e]
    # [chunks, T*P] -> [chunks, P, T] with edge t*P+p at [p, t]
    def shuffle(a):
        return np.ascontiguousarray(
            a.reshape(NCORES, chunks, T, P).transpose(0, 1, 3, 2))
    erow, ecol, evalv = shuffle(erow), shuffle(ecol), shuffle(evalv)

    in_maps = []
    for core in range(NCORES):
        in_maps.append({
            "feat": features[core * rows_per_core:(core + 1) * rows_per_core],
            "erow": erow[core], "ecol": ecol[core], "eval": evalv[core],
            "cA": cA_b, "cB": cB_b, "cC": cC_b, "bal": bal_b, "bias": bias_b,
            "gcnw": gcnw, "iota": iota, "ident": ident,
        })
    return in_maps, T


_BUILD_CACHE = {}


def kernel(features, adj_rows, adj_cols, adj_vals, down_k,
           tokens, wp_weight, global_token, pre_token_w, combine_w,
           balance_w, gcn_w, gcn_b):
    k = int(np.asarray(down_k))
    assert k == K_TOP, f"kernel hardcodes top-k={K_TOP}, got {k}"
    in_maps, T = prep_inputs(features, adj_rows, adj_cols, adj_vals, tokens,
                             wp_weight, global_token, pre_token_w, combine_w,
                             balance_w, gcn_w, gcn_b)
    if T not in _BUILD_CACHE:
        nc_new = build(T)
        _split_waits(nc_new)   # hardware-only fixup; breaks CoreSim if applied
        _BUILD_CACHE[T] = nc_new
    nc = _BUILD_CACHE[T]
    res = run_bass_kernel_spmd(nc, in_maps, list(range(NCORES)))
    out = np.concatenate([res.results[i]["out"] for i in range(NCORES)], axis=0)
    return out.astype(np.float32)

